# revision 13
# baseline (speedup 1.0000x reference)
"""Distributed AttentionBlock kernel for 8 TRN2 NeuronCores.

Sharding: tensor-parallel over heads (16 heads -> 2 per core) for
qkv-projection + attention; AllToAll redistributes attention output so
each core computes the out-projection for 512 tokens of each batch;
host-side unshard is a pure concat/interleave.

v4 changes vs v3:
  - qkv Q/K projections in fp8e4 DoubleRow (weights host-prescaled x32,
    rescaled in the bias pass); V path stays bf16
  - each batch's AllToAll split in two half-collectives (tokens of
    qc 0-3 / qc 4-7) with interleaved 256-token out-proj slices, so the
    second half's exchange+projection overlaps remaining attention and
    the tail shrinks to one 512KB collective + 2 token-blocks
  - head: group-0 x tiles DMA first; owT/outb loads deferred into the
    warmup so the first matmuls start ~6us earlier
  - O staging copies (PSUM->SBUF) moved to ScalarE (AF.Copy)
"""

import numpy as np

HIDDEN = 1024
HEAD_DIM = 64
N_CORES = 8
B = 2

# per-qc exp engine split over the 32 key blocks
ACT_BLOCKS = 19   # ScalarE: exact exp -> fp8e4
DVE_BLOCKS = 13   # DVE: int8 Schraudolph bit-exp -> fp8e4 bitcast

# Schraudolph fp8e4 bit-exp: int8(rne(s * A + B)) bitcast to fp8e4
# approximates exp(s/8).  A = 8*log2(e)*0.125; B = 56 - 0.0579*8
# (RMS-optimal piecewise-linear compensation, host-calibrated).
BITEXP8_A = 1.4426950408889634
BITEXP8_B = 55.54


def _exp_schedule():
    """Interleaved engine assignment for the 32 key blocks of one qc."""
    counts = {"act": ACT_BLOCKS, "dve": DVE_BLOCKS}
    counts = {k: v for k, v in counts.items() if v > 0}
    total = sum(counts.values())
    assert total == 32
    sched = []
    acc = {k: 0.0 for k in counts}
    for _ in range(32):
        for k in counts:
            acc[k] += counts[k] / total
        pick = max(acc, key=lambda k: acc[k])
        acc[pick] -= 1.0
        sched.append(pick)
    return sched


def build_nc(n_tok_b=4096, n_cores=8, hidden=1024):
    import concourse.bass as bass
    import concourse.bacc as bacc
    import concourse.tile as tile
    import concourse.mybir as mybir
    from concourse.masks import make_identity

    f32 = mybir.dt.float32
    bf16 = mybir.dt.bfloat16
    i8 = mybir.dt.int8
    f8 = mybir.dt.float8e4
    MPM = mybir.MatmulPerfMode
    AF = mybir.ActivationFunctionType
    ALU = mybir.AluOpType

    C = hidden
    CB = C // 128             # 8 contraction blocks
    assert CB == n_cores
    NB = n_tok_b              # tokens per batch
    T = B * NB
    QC = 512                  # query chunk
    NQC = NB // QC            # 8
    NMB = NB // 128           # 32 key blocks per batch
    GRP = 512                 # qkv token group
    NGRP = NB // GRP          # 8  (== NQC, used for interleaving)
    HSL = 256                 # out-proj half-slice tokens per core
    sched = _exp_schedule()

    nc = bacc.Bacc("TRN2", target_bir_lowering=False, debug=False,
                   num_devices=n_cores)

    xT_d = nc.declare_dram_parameter("xT", [CB, 128, T], bf16, isOutput=False)
    xT8_d = nc.declare_dram_parameter("xT8", [CB, 128, T], f8, isOutput=False)
    wT_d = nc.declare_dram_parameter("wT", [1, 128, CB * 128], bf16,
                                     isOutput=False)
    wT8_d = nc.declare_dram_parameter("wT8", [2, 128, CB * 128], f8,
                                      isOutput=False)
    qkvb_d = nc.declare_dram_parameter("qkvb", [3, 128, 1], f32,
                                       isOutput=False)
    owT_d = nc.declare_dram_parameter("owT", [CB, 128, C], bf16,
                                      isOutput=False)
    outb_d = nc.declare_dram_parameter("outb", [1, C], f32, isOutput=False)
    out_d = nc.declare_dram_parameter("out", [B * 2 * HSL, C], f32,
                                      isOutput=True)

    a2a_in = [[nc.dram_tensor(f"a2a_in{b}{h}", [n_cores, 128, HSL], bf16)
               for h in range(2)] for b in range(B)]
    a2a_out = [[nc.dram_tensor(f"a2a_out{b}{h}", [n_cores, 128, HSL], bf16)
                for h in range(2)] for b in range(B)]

    def tok0(j, h):
        """Token start (within a batch) of core j's half-h out-proj slice."""
        return (4 * h + j // 2) * 512 + (j % 2) * HSL

    with tile.TileContext(nc) as tc:
        with (
            tc.tile_pool(name="persist", bufs=1) as pp,
            tc.tile_pool(name="xt", bufs=2) as xtp,
            tc.tile_pool(name="pexp", bufs=6) as pexpp,
            tc.tile_pool(name="osbp", bufs=6) as osbp,
            tc.tile_pool(name="misc", bufs=2) as mp,
            tc.tile_pool(name="stp", bufs=3, space="PSUM") as stp,
            tc.tile_pool(name="ohp", bufs=2, space="PSUM") as ohp,
        ):
            # ---- tiles (allocation only; loads are sequenced below) ----
            ident = pp.tile([128, 128], bf16, tag="ident")
            wT = pp.tile([128, CB * 128], bf16, tag="wT")
            wTv = wT[:].rearrange("p (cb d) -> p cb d", cb=CB)
            wT8 = pp.tile([128, 2 * CB * 128], f8, tag="wT8")
            wT84 = wT8[:].rearrange("p (m cb d) -> p m cb d", m=2, cb=CB)
            owT = pp.tile([128, CB * C], bf16, tag="owT")
            owT3 = owT[:].rearrange("p (g co) -> p g co", co=C)
            bias_sb = pp.tile([128, 3], f32, tag="bias")
            outb_f = pp.tile([1, C], f32, tag="outbf")
            outb_sb = pp.tile([1, C], bf16, tag="outb")
            ones_sb = pp.tile([1, 128], bf16, tag="ones")
            qT = [pp.tile([128, NB], bf16, tag=f"qT{b}", name=f"qT{b}")
                  for b in range(B)]
            kT = [pp.tile([128, NB], bf16, tag=f"kT{b}", name=f"kT{b}")
                  for b in range(B)]
            V8 = [pp.tile([128, NMB * 144], f8, tag=f"V8{b}", name=f"V8{b}")
                  for b in range(B)]
            Oh0 = pp.tile([64, T], bf16, tag="Oh0")
            Oh1 = pp.tile([64, T], bf16, tag="Oh1")
            recvH = [[pp.tile([128, n_cores * HSL], bf16, tag=f"recv{b}{h}",
                              name=f"recv{b}{h}")
                      for h in range(2)] for b in range(B)]
            # normalization scratch: per (qc mod 4, head) slot
            rcb = pp.tile([1, 4 * QC], f32, tag="rcb")
            rcp = pp.tile([1, 4 * QC], f32, tag="rcp")
            rb = pp.tile([128, 4 * QC], f32, tag="rb")

            def setup_early():
                """Weight/bias loads needed by the first qkv group."""
                make_identity(nc, ident)
                nc.sync.dma_start(wT[:], wT_d[0])
                for m in range(2):
                    nc.sync.dma_start(
                        wT8[:, m * CB * 128:(m + 1) * CB * 128], wT8_d[m])
                for m in range(3):
                    nc.sync.dma_start(bias_sb[:, m:m + 1], qkvb_d[m])
                for b in range(B):
                    v84 = V8[b][:].rearrange("p (m d) -> p m d", d=144)
                    nc.vector.memset(v84[:, :, 64:65], 1.0)
                    nc.vector.memset(v84[:, :, 129:130], 1.0)

            def setup_late():
                """Out-proj weights: not needed until ~300us in."""
                for g in range(CB):
                    nc.sync.dma_start(owT3[:, g], owT_d[g])
                nc.sync.dma_start(outb_f[:], outb_d[:])
                nc.vector.tensor_copy(outb_sb[:], outb_f[:])
                nc.vector.memset(ones_sb[:], 1.0)

            def qkv_group(b, grp):
                """qkv projection for 512 tokens of batch b (generator:
                yields between chunks so attention emission can interleave
                finely and keep the exp engines fed)."""
                t0 = b * NB + grp * GRP
                xt8 = xtp.tile([128, CB * GRP], f8, tag="xt8")
                xt83 = xt8[:].rearrange("p (cb t) -> p cb t", t=GRP)
                for cb in range(CB):
                    nc.sync.dma_start(xt83[:, cb], xT8_d[cb, :, t0:t0 + GRP])
                xt = xtp.tile([128, CB * GRP], bf16, tag="xt")
                xt3 = xt[:].rearrange("p (cb t) -> p cb t", t=GRP)
                for cb in range(CB):
                    nc.sync.dma_start(xt3[:, cb], xT_d[cb, :, t0:t0 + GRP])
                yield
                for m in range(3):
                    qp = stp.tile([128, 2 * GRP], f32, tag="st")
                    if m < 2:
                        # fp8 DoubleRow: 4 matmuls cover the 1024-contraction
                        for cbp in range(CB // 2):
                            nc.tensor.matmul(
                                qp[:, 0:GRP], wT84[:, m, 2 * cbp:2 * cbp + 2],
                                xt83[:, 2 * cbp:2 * cbp + 2],
                                start=(cbp == 0), stop=(cbp == CB // 2 - 1),
                                perf_mode=MPM.DoubleRow)
                            if cbp == 1:
                                yield
                        dest = (qT if m == 0 else kT)[b][
                            :, grp * GRP:(grp + 1) * GRP]
                        nc.vector.tensor_scalar(dest, qp[:, 0:GRP],
                                                0.03125, bias_sb[:, m:m + 1],
                                                op0=ALU.mult, op1=ALU.add)
                        yield
                    else:
                        for cb in range(CB):
                            nc.tensor.matmul(qp[:, 0:GRP], wTv[:, cb],
                                             xt3[:, cb], start=(cb == 0),
                                             stop=(cb == CB - 1))
                            if cb == 3:
                                yield
                        vs = mp.tile([128, GRP], bf16, tag="vs")
                        nc.vector.tensor_scalar(vs[:], qp[:, 0:GRP],
                                                bias_sb[:, 2:3],
                                                None, op0=ALU.add)
                        tp = stp.tile([128, 2 * GRP], bf16, tag="st")
                        for j in range(GRP // 128):
                            nc.tensor.transpose(
                                tp[:, j * 128:(j + 1) * 128],
                                vs[:, j * 128:(j + 1) * 128], ident[:])
                        mb0 = grp * (GRP // 128)
                        v84 = V8[b][:].rearrange("p (m d) -> p m d", d=144)
                        tp3 = tp[:, 0:GRP].rearrange("p (j a) -> p j a", a=128)
                        nc.vector.tensor_copy(v84[:, mb0:mb0 + 4, 0:64],
                                              tp3[:, :, 0:64])
                        nc.vector.tensor_copy(v84[:, mb0:mb0 + 4, 65:129],
                                              tp3[:, :, 64:128])
                        yield

            def attention_qc(b, qc, fins_out, filler=None):
                """S + exp + PV for one 512-query chunk (generator: yields
                after each of the 16 slots).  Deferred normalize closures
                are appended to fins_out.  `filler` is an optional
                generator stepped at every slot."""
                qsl = slice(qc * QC, (qc + 1) * QC)
                oh0 = ohp.tile([65, QC], f32, tag="oh")
                oh1 = ohp.tile([65, QC], f32, tag="oh")
                V83 = V8[b][:].rearrange("p (m d) -> p m d", d=144)

                def s_one(mb):
                    st = stp.tile([128, 2 * QC], f32, tag="st")
                    nc.tensor.matmul(st[:, 0:QC],
                                     kT[b][0:64, mb * 128:mb * 128 + 128],
                                     qT[b][0:64, qsl],
                                     start=True, stop=True)
                    nc.tensor.matmul(st[:, QC:2 * QC],
                                     kT[b][64:128, mb * 128:mb * 128 + 128],
                                     qT[b][64:128, qsl],
                                     start=True, stop=True)
                    return st

                def exp_block(st, pe8, half, eng):
                    dst = pe8[:, half * 2 * QC:(half + 1) * 2 * QC]
                    if eng == "act":
                        nc.scalar.activation(dst, st[:], AF.Exp, scale=0.125)
                    else:
                        nc.vector.tensor_scalar(dst.bitcast(i8), st[:],
                                                BITEXP8_A, BITEXP8_B,
                                                op0=ALU.mult, op1=ALU.add)

                def pv_dr(slot, pe8):
                    """DoubleRow PV covering key blocks 2*slot, 2*slot+1."""
                    pe83 = pe8[:].rearrange("p (m q) -> p m q", q=2 * QC)
                    first = (slot == 0)
                    last = (slot == 15)
                    nc.tensor.matmul(
                        oh0[:], V83[:, 2 * slot:2 * slot + 2, 0:65],
                        pe83[:, :, 0:QC],
                        start=first, stop=last, perf_mode=MPM.DoubleRow)
                    nc.tensor.matmul(
                        oh1[:], V83[:, 2 * slot:2 * slot + 2, 65:130],
                        pe83[:, :, QC:2 * QC],
                        start=first, stop=last, perf_mode=MPM.DoubleRow)

                def step_filler():
                    if filler is not None:
                        try:
                            next(filler)
                        except StopIteration:
                            pass

                sts = [s_one(0), s_one(1)]
                for slot in range(16):
                    st0, st1 = sts
                    if slot + 1 < 16:
                        sts = [s_one(2 * slot + 2), s_one(2 * slot + 3)]
                    step_filler()
                    pe8 = pexpp.tile([128, 4 * QC], f8, tag="pe")
                    exp_block(st0, pe8, 0, sched[2 * slot])
                    exp_block(st1, pe8, 1, sched[2 * slot + 1])
                    pv_dr(slot, pe8)
                    yield
                if filler is not None:
                    for _ in filler:
                        pass
                # stage O+den to SBUF immediately (frees the oh PSUM
                # banks); the normalize is returned as a deferred closure
                # so it can be scheduled off the critical path.
                for h, oh in ((0, oh0), (1, oh1)):
                    sl = slice(((qc % 2) * 2 + h) * QC,
                               ((qc % 2) * 2 + h + 1) * QC)
                    osb = osbp.tile([65, QC], f32, tag="osb")
                    nc.scalar.activation(osb[:], oh[:], AF.Copy)
                    nc.sync.dma_start(rcb[0:1, sl], osb[64:65, :])
                    dest = (Oh0 if h == 0 else Oh1)[
                        :, b * NB + qc * QC: b * NB + (qc + 1) * QC]

                    def fin(sl=sl, osb=osb, dest=dest):
                        nc.vector.reciprocal_approx_fast(rcp[0:1, sl],
                                                         rcb[0:1, sl])
                        nc.gpsimd.partition_broadcast(rb[0:64, sl],
                                                      rcp[0:1, sl])
                        nc.vector.scalar_tensor_tensor(
                            dest, osb[0:64, :], 1.0, rb[0:64, sl],
                            op0=ALU.mult, op1=ALU.mult)
                    fins_out.append(fin)

            def a2a_launch(b, h):
                for j in range(n_cores):
                    t0 = b * NB + tok0(j, h)
                    nc.sync.dma_start(a2a_in[b][h][j, 0:64, :],
                                      Oh0[:, t0:t0 + HSL])
                    nc.sync.dma_start(a2a_in[b][h][j, 64:128, :],
                                      Oh1[:, t0:t0 + HSL])
                nc.gpsimd.collective_compute(
                    "AllToAll", ALU.bypass,
                    replica_groups=[list(range(n_cores))],
                    ins=[a2a_in[b][h].ap().opt()],
                    outs=[a2a_out[b][h].ap().opt()],
                )

            def a2a_recv(b, h):
                for g in range(n_cores):
                    nc.sync.dma_start(
                        recvH[b][h][:, g * HSL:(g + 1) * HSL],
                        a2a_out[b][h][g])

            def outproj_tb(b, h, tb):
                """out projection for 128 tokens of my half-h slice."""
                recv3 = recvH[b][h][:].rearrange("p (g t) -> p g t", t=HSL)
                ot = mp.tile([128, C], f32, tag="ot")
                for co2 in range(C // 512):
                    pj = stp.tile([128, 2 * QC], f32, tag="st")
                    for g in range(n_cores):
                        nc.tensor.matmul(
                            pj[:, 0:512],
                            recv3[:, g, tb * 128:tb * 128 + 128],
                            owT3[:, g, co2 * 512:(co2 + 1) * 512],
                            start=(g == 0), stop=False)
                    nc.tensor.matmul(pj[:, 0:512], ones_sb[:],
                                     outb_sb[:, co2 * 512:(co2 + 1) * 512],
                                     start=False, stop=True)
                    nc.vector.tensor_copy(ot[:, co2 * 512:(co2 + 1) * 512],
                                          pj[:, 0:512])
                r0 = b * 2 * HSL + h * HSL + tb * 128
                nc.sync.dma_start(out_d[r0:r0 + 128, :], ot[:])

            # ================= pipeline =================
            from itertools import chain

            def drain(g):
                for _ in g:
                    pass

            pending = []

            def flush_pending(n=None):
                k = len(pending) if n is None else n
                for _ in range(k):
                    if pending:
                        pending.pop(0)()

            # group-0 x tiles first in the DMA queue, then weights, so the
            # first matmuls start as early as possible.
            g0 = [qkv_group(0, g) for g in range(NGRP)]
            next(g0[0])
            setup_early()
            drain(g0[0])
            drain(g0[1])
            att = attention_qc(0, 0, pending)
            done = 0
            for g in range(2, NGRP):
                if g == 3:
                    setup_late()
                alive = True
                tick = 0
                while alive:
                    try:
                        next(g0[g])
                    except StopIteration:
                        alive = False
                    tick += 1
                    if tick % 2 == 0 and done < min(2 * g - 1, 16):
                        next(att)
                        done += 1
            drain(att)
            flush_pending(2)
            for qc in range(1, NQC):
                filler = (chain(qkv_group(1, qc - 1), qkv_group(1, NGRP - 1))
                          if qc == NQC - 1 else qkv_group(1, qc - 1))
                drain(attention_qc(0, qc, pending, filler=filler))
                flush_pending(2)
                if qc == 3:
                    flush_pending()
                    a2a_launch(0, 0)
                if qc == 5:
                    a2a_recv(0, 0)
            flush_pending()
            a2a_launch(0, 1)
            for qc in range(NQC):
                drain(attention_qc(1, qc, pending))
                if qc >= 1:
                    flush_pending(2)
                if qc == 1:
                    outproj_tb(0, 0, 0)
                if qc == 2:
                    a2a_recv(0, 1)
                    outproj_tb(0, 0, 1)
                if qc == 3:
                    flush_pending()
                    a2a_launch(1, 0)
                    outproj_tb(0, 1, 0)
                if qc == 4:
                    outproj_tb(0, 1, 1)
                if qc == 5:
                    a2a_recv(1, 0)
                if qc == 6:
                    outproj_tb(1, 0, 0)
                if qc == 7:
                    outproj_tb(1, 0, 1)
            flush_pending()
            a2a_launch(1, 1)
            a2a_recv(1, 1)
            outproj_tb(1, 1, 0)
            outproj_tb(1, 1, 1)

    nc.compile()
    return nc


def shard_inputs(x, qkv_w, qkv_b, out_w, out_b, n_cores=8):
    """Per-core input maps with host-side transpose + bf16/fp8 cast."""
    import ml_dtypes
    bf = ml_dtypes.bfloat16
    f8 = ml_dtypes.float8_e4m3fn
    Bv, N, C = x.shape
    T = Bv * N
    CB = C // 128
    # xT [CB, 128, T]
    xr = x.reshape(T, CB, 128).transpose(1, 2, 0)
    xT = np.ascontiguousarray(xr.astype(bf))
    xT8 = np.ascontiguousarray(xr.astype(f8))
    # owT [CB, 128, C]: owT[cb, p, co] = out_w[co, cb*128+p]
    owT = np.ascontiguousarray(
        out_w.astype(bf).T.reshape(CB, 128, C))
    outb = np.ascontiguousarray(out_b.reshape(1, C).astype(np.float32))
    in_maps = []
    for c in range(n_cores):
        r0 = c * 128
        # wT [m, 128, CB*128]: wT[m, p, cb*128+d] = qkv_w[m*C+r0+d, cb*128+p]
        w = np.stack([qkv_w[m * C + r0: m * C + r0 + 128] for m in range(3)])
        wt = (w.reshape(3, 128, CB, 128).transpose(0, 3, 2, 1)
              .reshape(3, 128, CB * 128))
        wT = np.ascontiguousarray(wt[2:3].astype(bf))
        wT8 = np.ascontiguousarray((wt[0:2] * 32.0).astype(f8))
        bvec = np.stack([qkv_b[m * C + r0: m * C + r0 + 128]
                         for m in range(3)])[:, :, None]
        in_maps.append({
            "xT": xT,
            "xT8": xT8,
            "wT": wT,
            "wT8": wT8,
            "qkvb": np.ascontiguousarray(bvec.astype(np.float32)),
            "owT": owT,
            "outb": outb,
        })
    return in_maps


def unshard(results, Bv, N, C, n_cores=8):
    """results[c]["out"] is [B*512, C]: per batch, two 256-token half
    slices (from qc c//2 and qc 4+c//2, 256-half c%2)."""
    HSL = 256
    out = np.empty((Bv, N, C), dtype=np.float32)
    for c in range(n_cores):
        o = results[c]["out"]
        for b in range(Bv):
            for h in range(2):
                t0 = (4 * h + c // 2) * 512 + (c % 2) * HSL
                out[b, t0:t0 + HSL, :] = \
                    o[b * 2 * HSL + h * HSL: b * 2 * HSL + (h + 1) * HSL]
    return out


_NC_CACHE = {}


def kernel(x, qkv_w, qkv_b, out_w, out_b):
    from concourse import bass_utils
    x = np.asarray(x)
    Bv, N, C = x.shape
    key = (N, C)
    if key not in _NC_CACHE:
        _NC_CACHE[key] = build_nc(n_tok_b=N, n_cores=N_CORES, hidden=C)
    nc = _NC_CACHE[key]
    in_maps = shard_inputs(x, np.asarray(qkv_w), np.asarray(qkv_b),
                           np.asarray(out_w), np.asarray(out_b),
                           n_cores=N_CORES)
    res = bass_utils.run_bass_kernel_spmd(nc, in_maps,
                                          core_ids=list(range(N_CORES)))
    return unshard(res.results, Bv, N, C, n_cores=N_CORES)


# revision 19
# speedup vs baseline: 1.0175x; 1.0175x over previous
"""Distributed AttentionBlock kernel for 8 TRN2 NeuronCores.

Sharding: tensor-parallel over heads (16 heads -> 2 per core) for
qkv-projection + attention; AllToAll redistributes attention output so
each core computes the out-projection for 512 tokens of each batch;
host-side unshard is a pure concat/interleave.

v4 changes vs v3:
  - qkv Q/K projections in fp8e4 DoubleRow (weights host-prescaled x32,
    rescaled in the bias pass); V path stays bf16
  - each batch's AllToAll split in two half-collectives (tokens of
    qc 0-3 / qc 4-7) with interleaved 256-token out-proj slices, so the
    second half's exchange+projection overlaps remaining attention and
    the tail shrinks to one 512KB collective + 2 token-blocks
  - head: group-0 x tiles DMA first; owT/outb loads deferred into the
    warmup so the first matmuls start ~6us earlier
  - O staging copies (PSUM->SBUF) moved to ScalarE (AF.Copy)
"""

import numpy as np

HIDDEN = 1024
HEAD_DIM = 64
N_CORES = 8
B = 2

# per-qc exp engine split over the 32 key blocks
ACT_BLOCKS = 19   # ScalarE: exact exp -> fp8e4
DVE_BLOCKS = 13   # DVE: int8 Schraudolph bit-exp -> fp8e4 bitcast

# Schraudolph fp8e4 bit-exp: int8(rne(s * A + B)) bitcast to fp8e4
# approximates exp(s/8).  A = 8*log2(e)*0.125; B = 56 - 0.0579*8
# (RMS-optimal piecewise-linear compensation, host-calibrated).
BITEXP8_A = 1.4426950408889634
BITEXP8_B = 55.54


def _exp_schedule():
    """Interleaved engine assignment for the 32 key blocks of one qc."""
    counts = {"act": ACT_BLOCKS, "dve": DVE_BLOCKS}
    counts = {k: v for k, v in counts.items() if v > 0}
    total = sum(counts.values())
    assert total == 32
    sched = []
    acc = {k: 0.0 for k in counts}
    for _ in range(32):
        for k in counts:
            acc[k] += counts[k] / total
        pick = max(acc, key=lambda k: acc[k])
        acc[pick] -= 1.0
        sched.append(pick)
    return sched


def build_nc(n_tok_b=4096, n_cores=8, hidden=1024):
    import concourse.bass as bass
    import concourse.bacc as bacc
    import concourse.tile as tile
    import concourse.mybir as mybir
    from concourse.masks import make_identity

    f32 = mybir.dt.float32
    bf16 = mybir.dt.bfloat16
    i8 = mybir.dt.int8
    f8 = mybir.dt.float8e4
    MPM = mybir.MatmulPerfMode
    AF = mybir.ActivationFunctionType
    ALU = mybir.AluOpType

    C = hidden
    CB = C // 128             # 8 contraction blocks
    assert CB == n_cores
    NB = n_tok_b              # tokens per batch
    T = B * NB
    QC = 512                  # query chunk
    NQC = NB // QC            # 8
    NMB = NB // 128           # 32 key blocks per batch
    GRP = 512                 # qkv token group
    NGRP = NB // GRP          # 8  (== NQC, used for interleaving)
    HSL = 256                 # out-proj half-slice tokens per core
    sched = _exp_schedule()

    nc = bacc.Bacc("TRN2", target_bir_lowering=False, debug=False,
                   num_devices=n_cores)

    xT_d = nc.declare_dram_parameter("xT", [CB, 128, T], bf16, isOutput=False)
    xT8_d = nc.declare_dram_parameter("xT8", [CB, 128, T], f8, isOutput=False)
    wT_d = nc.declare_dram_parameter("wT", [1, 128, CB * 128], bf16,
                                     isOutput=False)
    wT8_d = nc.declare_dram_parameter("wT8", [2, 128, CB * 128], f8,
                                      isOutput=False)
    qkvb_d = nc.declare_dram_parameter("qkvb", [3, 128, 1], f32,
                                       isOutput=False)
    owT_d = nc.declare_dram_parameter("owT", [CB, 128, C], bf16,
                                      isOutput=False)
    outb_d = nc.declare_dram_parameter("outb", [1, C], f32, isOutput=False)
    out_d = nc.declare_dram_parameter("out", [B * 2 * HSL, C], f32,
                                      isOutput=True)

    a2a_in = [[nc.dram_tensor(f"a2a_in{b}{h}", [n_cores, 128, HSL], bf16)
               for h in range(2)] for b in range(B)]
    a2a_out = [[nc.dram_tensor(f"a2a_out{b}{h}", [n_cores, 128, HSL], bf16)
                for h in range(2)] for b in range(B)]

    def tok0(j, h):
        """Token start (within a batch) of core j's half-h out-proj slice."""
        return (4 * h + j // 2) * 512 + (j % 2) * HSL

    with tile.TileContext(nc) as tc:
        with (
            tc.tile_pool(name="persist", bufs=1) as pp,
            tc.tile_pool(name="xt", bufs=2) as xtp,
            tc.tile_pool(name="pexp", bufs=6) as pexpp,
            tc.tile_pool(name="osbp", bufs=6) as osbp,
            tc.tile_pool(name="misc", bufs=2) as mp,
            tc.tile_pool(name="stp", bufs=3, space="PSUM") as stp,
            tc.tile_pool(name="ohp", bufs=2, space="PSUM") as ohp,
        ):
            # ---- tiles (allocation only; loads are sequenced below) ----
            ident = pp.tile([128, 128], bf16, tag="ident")
            wT = pp.tile([128, CB * 128], bf16, tag="wT")
            wTv = wT[:].rearrange("p (cb d) -> p cb d", cb=CB)
            wT8 = pp.tile([128, 2 * CB * 128], f8, tag="wT8")
            wT84 = wT8[:].rearrange("p (m cb d) -> p m cb d", m=2, cb=CB)
            owT = pp.tile([128, CB * C], bf16, tag="owT")
            owT3 = owT[:].rearrange("p (g co) -> p g co", co=C)
            bias_sb = pp.tile([128, 3], f32, tag="bias")
            outb_f = pp.tile([1, C], f32, tag="outbf")
            outb_sb = pp.tile([1, C], bf16, tag="outb")
            ones_sb = pp.tile([1, 128], bf16, tag="ones")
            qT = [pp.tile([128, NB], bf16, tag=f"qT{b}", name=f"qT{b}")
                  for b in range(B)]
            kT = [pp.tile([128, NB], bf16, tag=f"kT{b}", name=f"kT{b}")
                  for b in range(B)]
            V8 = [pp.tile([128, NMB * 144], f8, tag=f"V8{b}", name=f"V8{b}")
                  for b in range(B)]
            Oh0 = pp.tile([64, T], bf16, tag="Oh0")
            Oh1 = pp.tile([64, T], bf16, tag="Oh1")
            recvH = [[pp.tile([128, n_cores * HSL], bf16, tag=f"recv{b}{h}",
                              name=f"recv{b}{h}")
                      for h in range(2)] for b in range(B)]
            # normalization scratch: per (qc mod 4, head) slot
            rcb = pp.tile([1, 4 * QC], f32, tag="rcb")
            rcp = pp.tile([1, 4 * QC], f32, tag="rcp")
            rb = pp.tile([128, 4 * QC], f32, tag="rb")

            def setup_early():
                """Weight/bias loads needed by the first qkv group (on the
                scalar DMA queue so they don't serialize behind the x
                tiles on the sync queue)."""
                make_identity(nc, ident)
                for m in range(2):
                    nc.scalar.dma_start(
                        wT8[:, m * CB * 128:(m + 1) * CB * 128], wT8_d[m])
                nc.scalar.dma_start(wT[:], wT_d[0])
                for m in range(3):
                    nc.scalar.dma_start(bias_sb[:, m:m + 1], qkvb_d[m])
                for b in range(B):
                    v84 = V8[b][:].rearrange("p (m d) -> p m d", d=144)
                    nc.vector.memset(v84[:, :, 64:65], 1.0)
                    nc.vector.memset(v84[:, :, 129:130], 1.0)

            def setup_late():
                """Out-proj weights: not needed until ~300us in."""
                for g in range(CB):
                    nc.scalar.dma_start(owT3[:, g], owT_d[g])
                nc.scalar.dma_start(outb_f[:], outb_d[:])
                nc.vector.tensor_copy(outb_sb[:], outb_f[:])
                nc.vector.memset(ones_sb[:], 1.0)

            def qkv_group(b, grp):
                """qkv projection for 512 tokens of batch b (generator:
                yields between chunks so attention emission can interleave
                finely and keep the exp engines fed)."""
                t0 = b * NB + grp * GRP
                xt8 = xtp.tile([128, CB * GRP], f8, tag="xt8")
                xt83 = xt8[:].rearrange("p (cb t) -> p cb t", t=GRP)
                for cb in range(CB):
                    nc.sync.dma_start(xt83[:, cb], xT8_d[cb, :, t0:t0 + GRP])
                xt = xtp.tile([128, CB * GRP], bf16, tag="xt")
                xt3 = xt[:].rearrange("p (cb t) -> p cb t", t=GRP)
                for cb in range(CB):
                    # bf16 x tiles on the gpsimd DMA queue: halves the
                    # per-group sync-queue occupancy
                    nc.gpsimd.dma_start(xt3[:, cb], xT_d[cb, :, t0:t0 + GRP])
                yield
                for m in range(3):
                    qp = stp.tile([128, 2 * GRP], f32, tag="st")
                    if m < 2:
                        # fp8 DoubleRow: 4 matmuls cover the 1024-contraction
                        for cbp in range(CB // 2):
                            nc.tensor.matmul(
                                qp[:, 0:GRP], wT84[:, m, 2 * cbp:2 * cbp + 2],
                                xt83[:, 2 * cbp:2 * cbp + 2],
                                start=(cbp == 0), stop=(cbp == CB // 2 - 1),
                                perf_mode=MPM.DoubleRow)
                            if cbp == 1:
                                yield
                        # yield BEFORE the bias pass: it enters the DVE
                        # queue a slot later, when the matmuls it waits on
                        # are already done (no DVE head-of-line stall)
                        yield
                        dest = (qT if m == 0 else kT)[b][
                            :, grp * GRP:(grp + 1) * GRP]
                        nc.vector.tensor_scalar(dest, qp[:, 0:GRP],
                                                0.03125, bias_sb[:, m:m + 1],
                                                op0=ALU.mult, op1=ALU.add)
                        yield
                    else:
                        for cb in range(CB):
                            nc.tensor.matmul(qp[:, 0:GRP], wTv[:, cb],
                                             xt3[:, cb], start=(cb == 0),
                                             stop=(cb == CB - 1))
                            if cb == 3:
                                yield
                        yield
                        vs = mp.tile([128, GRP], bf16, tag="vs")
                        nc.vector.tensor_scalar(vs[:], qp[:, 0:GRP],
                                                bias_sb[:, 2:3],
                                                None, op0=ALU.add)
                        yield
                        tp = stp.tile([128, 2 * GRP], bf16, tag="st")
                        for j in range(GRP // 128):
                            nc.tensor.transpose(
                                tp[:, j * 128:(j + 1) * 128],
                                vs[:, j * 128:(j + 1) * 128], ident[:])
                        yield
                        mb0 = grp * (GRP // 128)
                        v84 = V8[b][:].rearrange("p (m d) -> p m d", d=144)
                        tp3 = tp[:, 0:GRP].rearrange("p (j a) -> p j a", a=128)
                        nc.vector.tensor_copy(v84[:, mb0:mb0 + 4, 0:64],
                                              tp3[:, :, 0:64])
                        nc.vector.tensor_copy(v84[:, mb0:mb0 + 4, 65:129],
                                              tp3[:, :, 64:128])
                        yield

            from collections import deque
            fillers = deque()

            def step_fill():
                while fillers:
                    try:
                        next(fillers[0])
                        return
                    except StopIteration:
                        fillers.popleft()

            def attention_qc(b, qc, fins_out, filler=None):
                """S + exp + PV for one 512-query chunk (generator: yields
                after each of the 16 slots).  Deferred normalize closures
                are appended to fins_out.  One step of the global filler
                deque is interleaved at every slot."""
                qsl = slice(qc * QC, (qc + 1) * QC)
                oh0 = ohp.tile([65, QC], f32, tag="oh")
                oh1 = ohp.tile([65, QC], f32, tag="oh")
                V83 = V8[b][:].rearrange("p (m d) -> p m d", d=144)

                def s_one(mb):
                    st = stp.tile([128, 2 * QC], f32, tag="st")
                    nc.tensor.matmul(st[:, 0:QC],
                                     kT[b][0:64, mb * 128:mb * 128 + 128],
                                     qT[b][0:64, qsl],
                                     start=True, stop=True)
                    nc.tensor.matmul(st[:, QC:2 * QC],
                                     kT[b][64:128, mb * 128:mb * 128 + 128],
                                     qT[b][64:128, qsl],
                                     start=True, stop=True)
                    return st

                def exp_block(st, pe8, half, eng):
                    dst = pe8[:, half * 2 * QC:(half + 1) * 2 * QC]
                    if eng == "act":
                        nc.scalar.activation(dst, st[:], AF.Exp, scale=0.125)
                    else:
                        nc.vector.tensor_scalar(dst.bitcast(i8), st[:],
                                                BITEXP8_A, BITEXP8_B,
                                                op0=ALU.mult, op1=ALU.add)

                def pv_dr(slot, pe8):
                    """DoubleRow PV covering key blocks 2*slot, 2*slot+1."""
                    pe83 = pe8[:].rearrange("p (m q) -> p m q", q=2 * QC)
                    first = (slot == 0)
                    last = (slot == 15)
                    nc.tensor.matmul(
                        oh0[:], V83[:, 2 * slot:2 * slot + 2, 0:65],
                        pe83[:, :, 0:QC],
                        start=first, stop=last, perf_mode=MPM.DoubleRow)
                    nc.tensor.matmul(
                        oh1[:], V83[:, 2 * slot:2 * slot + 2, 65:130],
                        pe83[:, :, QC:2 * QC],
                        start=first, stop=last, perf_mode=MPM.DoubleRow)

                sts = [s_one(0), s_one(1)]
                for slot in range(16):
                    st0, st1 = sts
                    if slot + 1 < 16:
                        sts = [s_one(2 * slot + 2), s_one(2 * slot + 3)]
                    step_fill()
                    pe8 = pexpp.tile([128, 4 * QC], f8, tag="pe")
                    exp_block(st0, pe8, 0, sched[2 * slot])
                    exp_block(st1, pe8, 1, sched[2 * slot + 1])
                    pv_dr(slot, pe8)
                    yield
                # stage O+den to SBUF immediately (frees the oh PSUM
                # banks); the normalize is returned as a deferred closure
                # so it can be scheduled off the critical path.
                for h, oh in ((0, oh0), (1, oh1)):
                    sl = slice(((qc % 2) * 2 + h) * QC,
                               ((qc % 2) * 2 + h + 1) * QC)
                    osb = osbp.tile([65, QC], f32, tag="osb")
                    nc.scalar.activation(osb[:], oh[:], AF.Copy)
                    nc.sync.dma_start(rcb[0:1, sl], osb[64:65, :])
                    dest = (Oh0 if h == 0 else Oh1)[
                        :, b * NB + qc * QC: b * NB + (qc + 1) * QC]

                    def fin(sl=sl, osb=osb, dest=dest):
                        nc.vector.reciprocal_approx_fast(rcp[0:1, sl],
                                                         rcb[0:1, sl])
                        nc.gpsimd.partition_broadcast(rb[0:64, sl],
                                                      rcp[0:1, sl])
                        nc.vector.scalar_tensor_tensor(
                            dest, osb[0:64, :], 1.0, rb[0:64, sl],
                            op0=ALU.mult, op1=ALU.mult)
                    fins_out.append(fin)

            def a2a_launch(b, h):
                for j in range(n_cores):
                    t0 = b * NB + tok0(j, h)
                    nc.sync.dma_start(a2a_in[b][h][j, 0:64, :],
                                      Oh0[:, t0:t0 + HSL])
                    nc.sync.dma_start(a2a_in[b][h][j, 64:128, :],
                                      Oh1[:, t0:t0 + HSL])
                nc.gpsimd.collective_compute(
                    "AllToAll", ALU.bypass,
                    replica_groups=[list(range(n_cores))],
                    ins=[a2a_in[b][h].ap().opt()],
                    outs=[a2a_out[b][h].ap().opt()],
                )

            def a2a_recv(b, h):
                for g in range(n_cores):
                    nc.sync.dma_start(
                        recvH[b][h][:, g * HSL:(g + 1) * HSL],
                        a2a_out[b][h][g])

            def outproj_tb(b, h, tb):
                """out projection for 128 tokens of my half-h slice
                (generator: yields before each PSUM->SBUF copy so the copy
                enters the DVE queue after its matmuls have executed)."""
                recv3 = recvH[b][h][:].rearrange("p (g t) -> p g t", t=HSL)
                ot = mp.tile([128, C], f32, tag="ot")
                for co2 in range(C // 512):
                    pj = stp.tile([128, 2 * QC], f32, tag="st")
                    for g in range(n_cores):
                        nc.tensor.matmul(
                            pj[:, 0:512],
                            recv3[:, g, tb * 128:tb * 128 + 128],
                            owT3[:, g, co2 * 512:(co2 + 1) * 512],
                            start=(g == 0), stop=False)
                    nc.tensor.matmul(pj[:, 0:512], ones_sb[:],
                                     outb_sb[:, co2 * 512:(co2 + 1) * 512],
                                     start=False, stop=True)
                    yield
                    nc.vector.tensor_copy(ot[:, co2 * 512:(co2 + 1) * 512],
                                          pj[:, 0:512])
                    yield
                r0 = b * 2 * HSL + h * HSL + tb * 128
                nc.sync.dma_start(out_d[r0:r0 + 128, :], ot[:])

            # ================= pipeline =================
            def drain(g):
                for _ in g:
                    pass

            pending = []

            def flush_pending(n=None):
                k = len(pending) if n is None else n
                for _ in range(k):
                    if pending:
                        pending.pop(0)()

            # group-0 x tiles first in the DMA queues, weights on the
            # scalar queue in parallel, so the first matmuls start early.
            g0 = [qkv_group(0, g) for g in range(NGRP)]
            next(g0[0])
            setup_early()
            drain(g0[0])
            drain(g0[1])
            att = attention_qc(0, 0, pending)
            done = 0
            for g in range(2, NGRP):
                if g == 3:
                    setup_late()
                alive = True
                tick = 0
                while alive:
                    try:
                        next(g0[g])
                    except StopIteration:
                        alive = False
                    tick += 1
                    if tick % 2 == 0 and done < min(2 * g - 1, 16):
                        next(att)
                        done += 1
            drain(att)
            # ---- batch-0 attention; batch-1 qkv streams as filler ----
            for qc in range(1, NQC):
                flush_pending(2)
                if qc == 4:
                    flush_pending()
                    a2a_launch(0, 0)
                if qc == 6:
                    a2a_recv(0, 0)
                fillers.append(qkv_group(1, qc - 1))
                if qc == NQC - 1:
                    fillers.append(qkv_group(1, NQC - 1))
                drain(attention_qc(0, qc, pending))
            flush_pending()
            a2a_launch(0, 1)
            # ---- batch-1 attention; out-proj interleaves as filler ----
            for qc in range(NQC):
                flush_pending(2)
                if qc == 1:
                    fillers.append(outproj_tb(0, 0, 0))
                if qc == 2:
                    a2a_recv(0, 1)
                    fillers.append(outproj_tb(0, 0, 1))
                if qc == 3:
                    fillers.append(outproj_tb(0, 1, 0))
                if qc == 4:
                    flush_pending()
                    a2a_launch(1, 0)
                    fillers.append(outproj_tb(0, 1, 1))
                if qc == 6:
                    a2a_recv(1, 0)
                    fillers.append(outproj_tb(1, 0, 0))
                if qc == 7:
                    fillers.append(outproj_tb(1, 0, 1))
                drain(attention_qc(1, qc, pending))
            flush_pending()
            while fillers:
                drain(fillers.popleft())
            a2a_launch(1, 1)
            a2a_recv(1, 1)
            drain(outproj_tb(1, 1, 0))
            drain(outproj_tb(1, 1, 1))

    nc.compile()
    return nc


def shard_inputs(x, qkv_w, qkv_b, out_w, out_b, n_cores=8):
    """Per-core input maps with host-side transpose + bf16/fp8 cast."""
    import ml_dtypes
    bf = ml_dtypes.bfloat16
    f8 = ml_dtypes.float8_e4m3fn
    Bv, N, C = x.shape
    T = Bv * N
    CB = C // 128
    # xT [CB, 128, T]
    xr = x.reshape(T, CB, 128).transpose(1, 2, 0)
    xT = np.ascontiguousarray(xr.astype(bf))
    xT8 = np.ascontiguousarray(xr.astype(f8))
    # owT [CB, 128, C]: owT[cb, p, co] = out_w[co, cb*128+p]
    owT = np.ascontiguousarray(
        out_w.astype(bf).T.reshape(CB, 128, C))
    outb = np.ascontiguousarray(out_b.reshape(1, C).astype(np.float32))
    in_maps = []
    for c in range(n_cores):
        r0 = c * 128
        # wT [m, 128, CB*128]: wT[m, p, cb*128+d] = qkv_w[m*C+r0+d, cb*128+p]
        w = np.stack([qkv_w[m * C + r0: m * C + r0 + 128] for m in range(3)])
        wt = (w.reshape(3, 128, CB, 128).transpose(0, 3, 2, 1)
              .reshape(3, 128, CB * 128))
        wT = np.ascontiguousarray(wt[2:3].astype(bf))
        wT8 = np.ascontiguousarray((wt[0:2] * 32.0).astype(f8))
        bvec = np.stack([qkv_b[m * C + r0: m * C + r0 + 128]
                         for m in range(3)])[:, :, None]
        in_maps.append({
            "xT": xT,
            "xT8": xT8,
            "wT": wT,
            "wT8": wT8,
            "qkvb": np.ascontiguousarray(bvec.astype(np.float32)),
            "owT": owT,
            "outb": outb,
        })
    return in_maps


def unshard(results, Bv, N, C, n_cores=8):
    """results[c]["out"] is [B*512, C]: per batch, two 256-token half
    slices (from qc c//2 and qc 4+c//2, 256-half c%2)."""
    HSL = 256
    out = np.empty((Bv, N, C), dtype=np.float32)
    for c in range(n_cores):
        o = results[c]["out"]
        for b in range(Bv):
            for h in range(2):
                t0 = (4 * h + c // 2) * 512 + (c % 2) * HSL
                out[b, t0:t0 + HSL, :] = \
                    o[b * 2 * HSL + h * HSL: b * 2 * HSL + (h + 1) * HSL]
    return out


_NC_CACHE = {}


def kernel(x, qkv_w, qkv_b, out_w, out_b):
    from concourse import bass_utils
    x = np.asarray(x)
    Bv, N, C = x.shape
    key = (N, C)
    if key not in _NC_CACHE:
        _NC_CACHE[key] = build_nc(n_tok_b=N, n_cores=N_CORES, hidden=C)
    nc = _NC_CACHE[key]
    in_maps = shard_inputs(x, np.asarray(qkv_w), np.asarray(qkv_b),
                           np.asarray(out_w), np.asarray(out_b),
                           n_cores=N_CORES)
    res = bass_utils.run_bass_kernel_spmd(nc, in_maps,
                                          core_ids=list(range(N_CORES)))
    return unshard(res.results, Bv, N, C, n_cores=N_CORES)
